# revision 7
# baseline (speedup 1.0000x reference)
"""Masked batched dot-product attention on 8 Trainium2 NeuronCores (Bass/Tile).

Reference computation (per batch b):
    scores = Q @ K^T / sqrt(D)                  [Q, K]
    scores[:, k >= valid_len[b]] = -1e6
    attn   = softmax(scores, axis=-1)
    out    = attn @ V                           [Q, V]

Strategy:
  - Data-parallel over the batch dim: 32 batches -> 8 cores x 4 slots.
    Batches are assigned to (slot, core) sorted by valid_len so all cores
    run the same (SPMD) trace while each slot's K-extent is trimmed to the
    slot-wise max number of 128-wide K chunks.
  - Per (slot, k-chunk), transposed score layout [k, q]:
      scoresT = KT_chunk.T @ QT                  (PE, bf16, PSUM f32)
      expT    = exp(scoresT/sqrt(D) + bias[k])   (ScalarE -> SBUF bf16;
                bias is -1e9 on masked k so masked weights are exactly 0)
      O^T    += V_chunk.T-contraction of expT    (PE, accumulated in PSUM)
      acc    += expT                             (VectorE, bf16 2x mode)
  - O^T stays in PSUM f32 and is DMA'd straight to HBM (no DVE copy);
    acc [k_in_chunk, q] streams out per slot; the host finishes with
    sums = acc.sum(partition), out = (O^T).T / sums during unsharding.
  - AV matmuls are emitted one chunk late (software pipeline) so PE never
    waits on the current exp; dummy matmuls at t=0 pre-warm the PE clock
    (HAM) and a dummy exp pre-loads the ACT LUT table.
  - First chunk's kt is DMA'd separately and the first/last chunk's exp is
    split in Q-halves to shorten the lead-in and tail of the pipeline.
"""

import math

import ml_dtypes
import numpy as np

import concourse.tile as tile
import concourse.mybir as mybir
from concourse import bacc
from concourse.bass_utils import run_bass_kernel_spmd

F32 = mybir.dt.float32
BF16 = mybir.dt.bfloat16

B, Q, K, D, V = 32, 1024, 1024, 128, 128
N_CORES = 8
S = B // N_CORES          # batch slots per core
CH = 128                  # K-chunk size (PE contraction width)
NCH = K // CH             # max chunks
HALF = 512                # PSUM bank limit: 512 fp32 per matmul output
SCALE = 1.0 / math.sqrt(D)
NEG_BIAS = -1.0e9


def _build(n_chunks):
    """Build + compile the SPMD bass module for per-slot chunk counts."""
    nc = bacc.Bacc("TRN2", target_bir_lowering=False, debug=False,
                   num_devices=N_CORES)
    qt = nc.dram_tensor("qt", [S, D, Q], BF16, kind="ExternalInput")
    kt = nc.dram_tensor("kt", [S, D, K], BF16, kind="ExternalInput")
    vt = nc.dram_tensor("vt", [S, CH, NCH, V], BF16, kind="ExternalInput")
    mb = nc.dram_tensor("mbias", [S, CH, NCH], F32, kind="ExternalInput")
    ot = nc.dram_tensor("ot", [S, V, Q], BF16, kind="ExternalOutput")
    am = nc.dram_tensor("acc", [S, CH, Q], BF16, kind="ExternalOutput")

    Exp = mybir.ActivationFunctionType.Exp

    with tile.TileContext(nc) as tc:
        with (
            tc.tile_pool(name="io", bufs=2) as io,
            tc.tile_pool(name="consts", bufs=1) as consts,
            tc.tile_pool(name="expp", bufs=4) as expp,
            tc.tile_pool(name="accp", bufs=2) as accp,
            tc.tile_pool(name="outp", bufs=2) as outp,
            tc.tile_pool(name="ps_sc", bufs=2, space="PSUM") as ps_sc_pool,
            tc.tile_pool(name="ps_ot", bufs=2, space="PSUM") as ps_ot_pool,
        ):
            # Mask bias, per (k-partition, slot, chunk); SWDGE so it does not
            # take an HWDGE slot ahead of the critical first kt/qt loads.
            bias_t = consts.tile([CH, S, NCH], F32)
            nc.gpsimd.dma_start(out=bias_t, in_=mb.ap().rearrange("s p j -> p s j"))

            # Warm the PE clock with dependency-free dummy matmuls so the
            # first real matmuls run at full rate once input DMAs land.
            warm_w = consts.tile([CH, 1], BF16)
            nc.vector.memset(warm_w, 0.0)
            warm_x = consts.tile([CH, 256], BF16)
            nc.vector.memset(warm_x, 0.0)
            ps_warm = ps_ot_pool.tile([1, 256], F32, tag="ot", name="ps_warm")
            for _ in range(10):
                nc.tensor.matmul(ps_warm, lhsT=warm_w, rhs=warm_x,
                                 start=True, stop=True)
            # Pre-load the Exp LUT table set so the first real exp skips it.
            warm_e = consts.tile([CH, 1], BF16)
            nc.scalar.activation(warm_e, warm_x[:, 0:1], func=Exp)

            _a = sorted(range(S), key=lambda i: n_chunks[i])
            slot_order = _a[1:] + _a[:1]   # smallest slot last (short tail)
            first_s = slot_order[0]

            # ---- input DMAs, issued up-front on SP/HWDGE in slot order ----
            sb_kt0 = {}   # first chunk of kt  [D, CH]     (first slot only)
            sb_ktr = {}   # rest of kt         [D, n*CH]
            sb_qt = {}    # qt                 [D, Q] or two [D, HALF] halves
            sb_vt = {}    # vt                 [CH, n_c, V]
            for s in slot_order:
                n_c = n_chunks[s]
                if s == first_s:
                    # Critical path: first chunk's kt, then qt half 0.
                    kt0 = io.tile([D, CH], BF16, tag="kt0", name="kt0")
                    nc.sync.dma_start(out=kt0, in_=kt.ap()[s, :, 0:CH])
                    sb_kt0[s] = kt0
                    qt_h = []
                    for h in range(2):
                        qh = io.tile([D, HALF], BF16, tag=f"qth{h}", name=f"qth{h}")
                        nc.sync.dma_start(
                            out=qh, in_=qt.ap()[s, :, h * HALF:(h + 1) * HALF])
                        qt_h.append(qh)
                    sb_qt[s] = qt_h
                    if n_c > 1:
                        ktr = io.tile([D, (n_c - 1) * CH], BF16, tag="ktr",
                                      name="ktr")
                        nc.sync.dma_start(out=ktr, in_=kt.ap()[s, :, CH:n_c * CH])
                        sb_ktr[s] = ktr
                else:
                    qtf = io.tile([D, Q], BF16, tag="qt", name=f"qt{s}")
                    nc.sync.dma_start(out=qtf, in_=qt.ap()[s, :, :])
                    sb_qt[s] = qtf
                    ktf = io.tile([D, n_c * CH], BF16, tag="kt", name=f"kt{s}")
                    nc.sync.dma_start(out=ktf, in_=kt.ap()[s, :, 0:n_c * CH])
                    sb_ktr[s] = ktf
                vtt = io.tile([CH, n_c, V], BF16, tag="vt", name=f"vt{s}")
                nc.sync.dma_start(out=vtt, in_=vt.ap()[s, :, 0:n_c, :])
                sb_vt[s] = vtt

            # ---- main compute loop ----
            for si, s in enumerate(slot_order):
                n_c = n_chunks[s]
                first_slot = (si == 0)
                last_slot = (si == len(slot_order) - 1)

                def kt_chunk(c, s=s, first_slot=first_slot):
                    if first_slot:
                        if c == 0:
                            return sb_kt0[s]
                        return sb_ktr[s][:, (c - 1) * CH:c * CH]
                    return sb_ktr[s][:, c * CH:(c + 1) * CH]

                def qt_half(h, s=s, first_slot=first_slot):
                    if first_slot:
                        return sb_qt[s][h]
                    return sb_qt[s][:, h * HALF:(h + 1) * HALF]

                ps_ot = ps_ot_pool.tile([V, Q], F32, tag="ot", name=f"ot{s}")
                acc = accp.tile([CH, Q], BF16, tag="acc", name=f"acc{s}")
                exp_tiles = {}

                def emit_av(c, s=s, n_c=n_c, ps_ot=ps_ot, exp_tiles=exp_tiles):
                    e = exp_tiles.pop(c)
                    vj = sb_vt[s][:, c, :]
                    for h in range(2):
                        hs = slice(h * HALF, (h + 1) * HALF)
                        nc.tensor.matmul(ps_ot[:, hs], lhsT=vj, rhs=e[:, hs],
                                         start=(c == 0), stop=(c == n_c - 1))

                for c in range(n_c):
                    ps_sc = ps_sc_pool.tile([CH, Q], F32, tag="sc")
                    sb_exp = expp.tile([CH, Q], BF16, tag="e")
                    # Split the act in halves at the pipeline's lead (first
                    # real exp sooner) and tail (last AV + DMA sooner).
                    hsplit = (first_slot and c == 0) or \
                        (last_slot and c == n_c - 1)
                    for h in range(2):
                        hs = slice(h * HALF, (h + 1) * HALF)
                        nc.tensor.matmul(ps_sc[:, hs], lhsT=kt_chunk(c),
                                         rhs=qt_half(h), start=True, stop=True)
                        if hsplit:
                            nc.scalar.activation(
                                sb_exp[:, hs], ps_sc[:, hs], func=Exp,
                                bias=bias_t[:, s, c:c + 1], scale=SCALE)
                    if not hsplit:
                        nc.scalar.activation(
                            sb_exp, ps_sc, func=Exp,
                            bias=bias_t[:, s, c:c + 1], scale=SCALE)
                    exp_tiles[c] = sb_exp
                    # Denominator partials on DVE (bf16 2x mode).
                    if last_slot and c == n_c - 1:
                        for h in range(2):
                            hs = slice(h * HALF, (h + 1) * HALF)
                            if c == 0:
                                nc.vector.tensor_copy(acc[:, hs], sb_exp[:, hs])
                            else:
                                nc.vector.tensor_add(acc[:, hs], acc[:, hs],
                                                     sb_exp[:, hs])
                    elif c == 0:
                        nc.vector.tensor_copy(acc, sb_exp)
                    else:
                        nc.vector.tensor_add(acc, acc, sb_exp)
                    # AV matmuls one chunk late so PE never waits on the
                    # current chunk's exp.
                    if c >= 1:
                        emit_av(c - 1)
                emit_av(n_c - 1)

                # Outputs: O^T copied PSUM->SBUF bf16 on DVE (only PE/ACT/DVE
                # may touch PSUM). The last slot's DMAs go on SP/HWDGE
                # (shortest issue chain); earlier slots use SWDGE so their
                # sem-waits never block SP's input-DMA queue.
                out_eng = nc.sync if last_slot else nc.gpsimd
                for h in range(2):
                    hs = slice(h * HALF, (h + 1) * HALF)
                    sb_ot = outp.tile([V, HALF], BF16, tag=f"ot{h}")
                    nc.vector.tensor_copy(sb_ot, ps_ot[:, hs])
                    out_eng.dma_start(out=ot.ap()[s, :, hs], in_=sb_ot)
                nc.gpsimd.dma_start(out=am.ap()[s], in_=acc)
    nc.compile()
    return nc


_MODULE_CACHE = {}


def _get_module(n_chunks):
    key = tuple(n_chunks)
    if key not in _MODULE_CACHE:
        _MODULE_CACHE[key] = _build(key)
    return _MODULE_CACHE[key]


def _plan(L):
    """Assign batches to (slot, core) sorted by valid_len; per-slot chunk count."""
    order = np.argsort(L, kind="stable")
    grid = order.reshape(S, N_CORES)       # grid[s, c] = batch index
    n_chunks = tuple(
        max(1, int(math.ceil(int(L[grid[s, -1]]) / CH))) for s in range(S)
    )
    return grid, n_chunks


def _prepare_inputs(q, k, v, L, grid):
    kidx = np.arange(K).reshape(NCH, CH).T      # [CH, NCH] k index per (p, chunk)
    in_maps = []
    for c in range(N_CORES):
        bs = grid[:, c]
        qt_c = np.ascontiguousarray(q[bs].transpose(0, 2, 1)).astype(ml_dtypes.bfloat16)
        kt_c = np.ascontiguousarray(k[bs].transpose(0, 2, 1)).astype(ml_dtypes.bfloat16)
        # [S, K, V] -> [S, CH, NCH, V]: chunk c, in-chunk row p = k-index c*CH+p
        vt_c = np.ascontiguousarray(
            v[bs].reshape(S, NCH, CH, V).transpose(0, 2, 1, 3)
        ).astype(ml_dtypes.bfloat16)
        mb_c = np.empty((S, CH, NCH), np.float32)
        for s in range(S):
            mb_c[s] = np.where(kidx < int(L[grid[s, c]]), 0.0, NEG_BIAS)
        in_maps.append({"qt": qt_c, "kt": kt_c, "vt": vt_c, "mbias": mb_c})
    return in_maps


def _postprocess(results, grid):
    out = np.empty((B, Q, V), np.float32)
    for c in range(N_CORES):
        otc = results[c]["ot"].astype(np.float32)                # [S, V, Q]
        sums = results[c]["acc"].astype(np.float32).sum(axis=1)  # [S, Q]
        for s in range(S):
            b = grid[s, c]
            out[b] = (otc[s] / sums[s][None, :]).T
    return out


def kernel(**inputs):
    q = np.ascontiguousarray(np.asarray(inputs["queries"], dtype=np.float32))
    k = np.ascontiguousarray(np.asarray(inputs["keys"], dtype=np.float32))
    v = np.ascontiguousarray(np.asarray(inputs["values"], dtype=np.float32))
    L = np.clip(np.asarray(inputs["valid_lens"]).astype(np.int64).reshape(-1), 1, K)
    grid, n_chunks = _plan(L)
    nc = _get_module(n_chunks)
    in_maps = _prepare_inputs(q, k, v, L, grid)
    res = run_bass_kernel_spmd(nc, in_maps, core_ids=list(range(N_CORES)))
    return _postprocess(res.results, grid)


# revision 9
# speedup vs baseline: 1.0297x; 1.0297x over previous
"""Masked batched dot-product attention on 8 Trainium2 NeuronCores (Bass/Tile).

Reference computation (per batch b):
    scores = Q @ K^T / sqrt(D)                  [Q, K]
    scores[:, k >= valid_len[b]] = -1e6
    attn   = softmax(scores, axis=-1)
    out    = attn @ V                           [Q, V]

Strategy:
  - Data-parallel over the batch dim: 32 batches -> 8 cores x 4 slots.
    Batches are assigned to (slot, core) sorted by valid_len so all cores
    run the same (SPMD) trace while each slot's K-extent is trimmed to the
    slot-wise max number of 128-wide K chunks.
  - Per (slot, k-chunk), transposed score layout [k, q]:
      scoresT = KT_chunk.T @ QT                  (PE, bf16, PSUM f32)
      expT    = exp(scoresT/sqrt(D) + bias[k])   (ScalarE -> SBUF bf16;
                bias is -1e9 on masked k so masked weights are exactly 0)
      O^T    += V_chunk.T-contraction of expT    (PE, accumulated in PSUM)
      acc    += expT                             (VectorE, bf16 2x mode)
  - O^T stays in PSUM f32 and is DMA'd straight to HBM (no DVE copy);
    acc [k_in_chunk, q] streams out per slot; the host finishes with
    sums = acc.sum(partition), out = (O^T).T / sums during unsharding.
  - AV matmuls are emitted one chunk late (software pipeline) so PE never
    waits on the current exp; dummy matmuls at t=0 pre-warm the PE clock
    (HAM) and a dummy exp pre-loads the ACT LUT table.
  - First chunk's kt is DMA'd separately and the first/last chunk's exp is
    split in Q-halves to shorten the lead-in and tail of the pipeline.
"""

import math

import ml_dtypes
import numpy as np

import concourse.tile as tile
import concourse.mybir as mybir
from concourse import bacc
from concourse.bass_utils import run_bass_kernel_spmd

F32 = mybir.dt.float32
BF16 = mybir.dt.bfloat16

B, Q, K, D, V = 32, 1024, 1024, 128, 128
N_CORES = 8
S = B // N_CORES          # batch slots per core
CH = 128                  # K-chunk size (PE contraction width)
NCH = K // CH             # max chunks
HALF = 512                # PSUM bank limit: 512 fp32 per matmul output
SCALE = 1.0 / math.sqrt(D)
NEG_BIAS = -1.0e9


def _build(n_chunks):
    """Build + compile the SPMD bass module for per-slot chunk counts."""
    nc = bacc.Bacc("TRN2", target_bir_lowering=False, debug=False,
                   num_devices=N_CORES)
    qt = nc.dram_tensor("qt", [S, D, Q], BF16, kind="ExternalInput")
    kt = nc.dram_tensor("kt", [S, D, K], BF16, kind="ExternalInput")
    vt = nc.dram_tensor("vt", [S, CH, NCH, V], BF16, kind="ExternalInput")
    mb = nc.dram_tensor("mbias", [S, CH, NCH], F32, kind="ExternalInput")
    ot = nc.dram_tensor("ot", [S, V, Q], BF16, kind="ExternalOutput")
    am = nc.dram_tensor("acc", [S, CH, Q], BF16, kind="ExternalOutput")

    Exp = mybir.ActivationFunctionType.Exp

    with tile.TileContext(nc) as tc:
        with (
            tc.tile_pool(name="io", bufs=2) as io,
            tc.tile_pool(name="consts", bufs=1) as consts,
            tc.tile_pool(name="expp", bufs=4) as expp,
            tc.tile_pool(name="accp", bufs=2) as accp,
            tc.tile_pool(name="outp", bufs=2) as outp,
            tc.tile_pool(name="ps_sc", bufs=3, space="PSUM") as ps_sc_pool,
            tc.tile_pool(name="ps_ot", bufs=1, space="PSUM") as ps_ot_pool,
        ):
            # Mask bias, per (k-partition, slot, chunk); SWDGE so it does not
            # take an HWDGE slot ahead of the critical first kt/qt loads.
            bias_t = consts.tile([CH, S, NCH], F32)
            nc.gpsimd.dma_start(out=bias_t, in_=mb.ap().rearrange("s p j -> p s j"))

            # Warm the PE clock with dependency-free dummy matmuls so the
            # first real matmuls run at full rate once input DMAs land.
            warm_w = consts.tile([CH, 1], BF16)
            nc.vector.memset(warm_w, 0.0)
            warm_x = consts.tile([CH, 256], BF16)
            nc.vector.memset(warm_x, 0.0)
            ps_warm = ps_ot_pool.tile([1, 256], F32, tag="ot", name="ps_warm")
            for _ in range(10):
                nc.tensor.matmul(ps_warm, lhsT=warm_w, rhs=warm_x,
                                 start=True, stop=True)
            # Pre-load the Exp LUT table set so the first real exp skips it.
            warm_e = consts.tile([CH, 1], BF16)
            nc.scalar.activation(warm_e, warm_x[:, 0:1], func=Exp)

            _a = sorted(range(S), key=lambda i: n_chunks[i])
            slot_order = _a[1:] + _a[:1]   # smallest slot last (short tail)
            first_s = slot_order[0]

            # ---- input DMAs, issued up-front on SP/HWDGE in slot order ----
            sb_kt0 = {}   # first chunk of kt  [D, CH]     (first slot only)
            sb_ktr = {}   # rest of kt         [D, n*CH]
            sb_qt = {}    # qt                 [D, Q] or two [D, HALF] halves
            sb_vt = {}    # vt                 [CH, n_c, V]
            for s in slot_order:
                n_c = n_chunks[s]
                if s == first_s:
                    # Critical path: first chunk's kt, then qt half 0.
                    kt0 = io.tile([D, CH], BF16, tag="kt0", name="kt0")
                    nc.sync.dma_start(out=kt0, in_=kt.ap()[s, :, 0:CH])
                    sb_kt0[s] = kt0
                    qt_h = []
                    for h in range(2):
                        qh = io.tile([D, HALF], BF16, tag=f"qth{h}", name=f"qth{h}")
                        nc.sync.dma_start(
                            out=qh, in_=qt.ap()[s, :, h * HALF:(h + 1) * HALF])
                        qt_h.append(qh)
                    sb_qt[s] = qt_h
                    if n_c > 1:
                        ktr = io.tile([D, (n_c - 1) * CH], BF16, tag="ktr",
                                      name="ktr")
                        nc.sync.dma_start(out=ktr, in_=kt.ap()[s, :, CH:n_c * CH])
                        sb_ktr[s] = ktr
                else:
                    qtf = io.tile([D, Q], BF16, tag="qt", name=f"qt{s}")
                    nc.sync.dma_start(out=qtf, in_=qt.ap()[s, :, :])
                    sb_qt[s] = qtf
                    ktf = io.tile([D, n_c * CH], BF16, tag="kt", name=f"kt{s}")
                    nc.sync.dma_start(out=ktf, in_=kt.ap()[s, :, 0:n_c * CH])
                    sb_ktr[s] = ktf
                vtt = io.tile([CH, n_c, V], BF16, tag="vt", name=f"vt{s}")
                nc.sync.dma_start(out=vtt, in_=vt.ap()[s, :, 0:n_c, :])
                sb_vt[s] = vtt

            # ---- main compute loop ----
            for si, s in enumerate(slot_order):
                n_c = n_chunks[s]
                first_slot = (si == 0)
                last_slot = (si == len(slot_order) - 1)

                def kt_chunk(c, s=s, first_slot=first_slot):
                    if first_slot:
                        if c == 0:
                            return sb_kt0[s]
                        return sb_ktr[s][:, (c - 1) * CH:c * CH]
                    return sb_ktr[s][:, c * CH:(c + 1) * CH]

                def qt_half(h, s=s, first_slot=first_slot):
                    if first_slot:
                        return sb_qt[s][h]
                    return sb_qt[s][:, h * HALF:(h + 1) * HALF]

                ps_ot = ps_ot_pool.tile([V, Q], F32, tag="ot", name=f"ot{s}")
                acc = accp.tile([CH, Q], BF16, tag="acc", name=f"acc{s}")
                exp_tiles = {}

                def emit_av(c, s=s, n_c=n_c, ps_ot=ps_ot, exp_tiles=exp_tiles):
                    e = exp_tiles.pop(c)
                    vj = sb_vt[s][:, c, :]
                    for h in range(2):
                        hs = slice(h * HALF, (h + 1) * HALF)
                        nc.tensor.matmul(ps_ot[:, hs], lhsT=vj, rhs=e[:, hs],
                                         start=(c == 0), stop=(c == n_c - 1))

                for c in range(n_c):
                    ps_sc = ps_sc_pool.tile([CH, Q], F32, tag="sc")
                    sb_exp = expp.tile([CH, Q], BF16, tag="e")
                    tail = last_slot and c == n_c - 1
                    # Split the act in halves at the pipeline's lead (first
                    # real exp sooner) and tail (last AV + DMA sooner).
                    hsplit = (first_slot and c == 0) or tail
                    for h in range(2):
                        hs = slice(h * HALF, (h + 1) * HALF)
                        nc.tensor.matmul(ps_sc[:, hs], lhsT=kt_chunk(c),
                                         rhs=qt_half(h), start=True, stop=True)
                        if hsplit:
                            nc.scalar.activation(
                                sb_exp[:, hs], ps_sc[:, hs], func=Exp,
                                bias=bias_t[:, s, c:c + 1], scale=SCALE)
                    if not hsplit:
                        nc.scalar.activation(
                            sb_exp, ps_sc, func=Exp,
                            bias=bias_t[:, s, c:c + 1], scale=SCALE)
                    exp_tiles[c] = sb_exp
                    if not tail:
                        # Denominator partials on DVE (bf16 2x mode).
                        if c == 0:
                            nc.vector.tensor_copy(acc, sb_exp)
                        else:
                            nc.vector.tensor_add(acc, acc, sb_exp)
                        # AV matmuls one chunk late so PE never waits on the
                        # current chunk's exp.
                        if c >= 1:
                            emit_av(c - 1)
                    else:
                        # Whole-kernel tail: finest-grained dependency chain
                        # from the last exp to the last output DMA.
                        if c >= 1:
                            emit_av(c - 1)
                        emit_av(c)
                        h0, h1 = slice(0, HALF), slice(HALF, Q)
                        if c == 0:
                            nc.vector.tensor_copy(acc[:, h0], sb_exp[:, h0])
                        else:
                            nc.vector.tensor_add(acc[:, h0], acc[:, h0],
                                                 sb_exp[:, h0])
                        ot_h0 = outp.tile([V, HALF], BF16, tag="ot0")
                        nc.vector.tensor_copy(ot_h0, ps_ot[:, h0])
                        nc.sync.dma_start(out=am.ap()[s, :, h0], in_=acc[:, h0])
                        nc.sync.dma_start(out=ot.ap()[s, :, h0], in_=ot_h0)
                        if c == 0:
                            nc.vector.tensor_copy(acc[:, h1], sb_exp[:, h1])
                        else:
                            nc.vector.tensor_add(acc[:, h1], acc[:, h1],
                                                 sb_exp[:, h1])
                        nc.sync.dma_start(out=am.ap()[s, :, h1], in_=acc[:, h1])
                        ot_q = []
                        for qtr in range(2):
                            qs = slice(HALF + qtr * 256, HALF + (qtr + 1) * 256)
                            sb_q = outp.tile([V, 256], BF16, tag=f"otq{qtr}")
                            nc.vector.tensor_copy(sb_q, ps_ot[:, qs])
                            nc.sync.dma_start(out=ot.ap()[s, :, qs], in_=sb_q)

                if not last_slot:
                    emit_av(n_c - 1)
                    # O^T copied PSUM->SBUF bf16 on DVE (only PE/ACT/DVE may
                    # touch PSUM); SWDGE DMAs so their sem-waits never block
                    # SP's input-DMA queue.
                    for h in range(2):
                        hs = slice(h * HALF, (h + 1) * HALF)
                        sb_ot = outp.tile([V, HALF], BF16, tag=f"ot{h}")
                        nc.vector.tensor_copy(sb_ot, ps_ot[:, hs])
                        nc.gpsimd.dma_start(out=ot.ap()[s, :, hs], in_=sb_ot)
                    nc.gpsimd.dma_start(out=am.ap()[s], in_=acc)
    nc.compile()
    return nc


_MODULE_CACHE = {}


def _get_module(n_chunks):
    key = tuple(n_chunks)
    if key not in _MODULE_CACHE:
        _MODULE_CACHE[key] = _build(key)
    return _MODULE_CACHE[key]


def _plan(L):
    """Assign batches to (slot, core) sorted by valid_len; per-slot chunk count."""
    order = np.argsort(L, kind="stable")
    grid = order.reshape(S, N_CORES)       # grid[s, c] = batch index
    n_chunks = tuple(
        max(1, int(math.ceil(int(L[grid[s, -1]]) / CH))) for s in range(S)
    )
    return grid, n_chunks


def _prepare_inputs(q, k, v, L, grid):
    kidx = np.arange(K).reshape(NCH, CH).T      # [CH, NCH] k index per (p, chunk)
    in_maps = []
    for c in range(N_CORES):
        bs = grid[:, c]
        qt_c = np.ascontiguousarray(q[bs].transpose(0, 2, 1)).astype(ml_dtypes.bfloat16)
        kt_c = np.ascontiguousarray(k[bs].transpose(0, 2, 1)).astype(ml_dtypes.bfloat16)
        # [S, K, V] -> [S, CH, NCH, V]: chunk c, in-chunk row p = k-index c*CH+p
        vt_c = np.ascontiguousarray(
            v[bs].reshape(S, NCH, CH, V).transpose(0, 2, 1, 3)
        ).astype(ml_dtypes.bfloat16)
        mb_c = np.empty((S, CH, NCH), np.float32)
        for s in range(S):
            mb_c[s] = np.where(kidx < int(L[grid[s, c]]), 0.0, NEG_BIAS)
        in_maps.append({"qt": qt_c, "kt": kt_c, "vt": vt_c, "mbias": mb_c})
    return in_maps


def _postprocess(results, grid):
    out = np.empty((B, Q, V), np.float32)
    for c in range(N_CORES):
        otc = results[c]["ot"].astype(np.float32)                # [S, V, Q]
        sums = results[c]["acc"].astype(np.float32).sum(axis=1)  # [S, Q]
        for s in range(S):
            b = grid[s, c]
            out[b] = (otc[s] / sums[s][None, :]).T
    return out


def kernel(**inputs):
    q = np.ascontiguousarray(np.asarray(inputs["queries"], dtype=np.float32))
    k = np.ascontiguousarray(np.asarray(inputs["keys"], dtype=np.float32))
    v = np.ascontiguousarray(np.asarray(inputs["values"], dtype=np.float32))
    L = np.clip(np.asarray(inputs["valid_lens"]).astype(np.int64).reshape(-1), 1, K)
    grid, n_chunks = _plan(L)
    nc = _get_module(n_chunks)
    in_maps = _prepare_inputs(q, k, v, L, grid)
    res = run_bass_kernel_spmd(nc, in_maps, core_ids=list(range(N_CORES)))
    return _postprocess(res.results, grid)
